# revision 32
# baseline (speedup 1.0000x reference)
import sys
sys.path.insert(0, "/opt/trn_rl_repo")
import math
import numpy as np
import ml_dtypes

import concourse.bass as bass
from concourse import bacc, mybir
from concourse.tile import TileContext
from concourse.bass_utils import run_bass_kernel_spmd
from concourse.masks import make_identity

F32 = mybir.dt.float32
F32R = mybir.dt.float32r
F8 = mybir.dt.float8e4
AF = mybir.ActivationFunctionType
ALU = mybir.AluOpType
AX = mybir.AxisListType
DR = mybir.MatmulPerfMode.DoubleRow

N, G, E = 16384, 32, 524288
D, DFF, ZI, K, L = 512, 1024, 64, 4, 4
UMAP_A, UMAP_B = 1.577, 0.8951
BN_EPS = 1e-5
NCORES = 8
NL = N // NCORES      # 2048 local nodes per core
GL = G // NCORES      # 4 local graphs per core
NG = N // G           # 512 nodes per graph
KB2 = N // 256        # 64 paired source blocks (256 src nodes each)
RG = [list(range(NCORES))]

_NC_CACHE = None


def build_nc():
    nc = bacc.Bacc("TRN2", target_bir_lowering=False, debug=False,
                   enable_asserts=True, num_devices=NCORES)

    xt = nc.dram_tensor("xt", (10, NL), F32R, kind="ExternalInput")
    # full-graph x in paired fp8: cols = [x_hi(10) | x_lo(10) | ones(1) | pad]
    xg8 = nc.dram_tensor("xg8", (128, KB2, 2, 32), F8, kind="ExternalInput")
    # emb_w stacked for the hi/lo recombine: rows = [W, W, b, 0...]
    embwa = nc.dram_tensor("embwa", (32, D), F32R, kind="ExternalInput")
    acm8 = nc.dram_tensor("acm8", (128, KB2, 2, NL), F8, kind="ExternalInput")
    embw = nc.dram_tensor("embw", (10, D), F32R, kind="ExternalInput")
    gw1 = nc.dram_tensor("gw1", (L * D, D), F32R, kind="ExternalInput")
    gw2 = nc.dram_tensor("gw2", (L * D, D), F32R, kind="ExternalInput")
    mw1 = nc.dram_tensor("mw1", (D, DFF), F32R, kind="ExternalInput")
    mw2 = nc.dram_tensor("mw2", (DFF, DFF), F32R, kind="ExternalInput")
    mw3 = nc.dram_tensor("mw3", (DFF, ZI), F32R, kind="ExternalInput")
    # head weights batched in pairs: hw1b[64*kp:64*kp+64] = [w1_{2kp}|w1_{2kp+1}]
    hw1b = nc.dram_tensor("hw1b", (2 * ZI, 2 * ZI), F32R, kind="ExternalInput")
    # per-head w2 with bias row appended: hw2q[65k:65k+64] = w2_k, row 65k+64 = b2_k
    hw2q = nc.dram_tensor("hw2q", (K * (ZI + 1), ZI), F32R,
                          kind="ExternalInput")
    # per-head h1 bias columns
    hb1q_d = nc.dram_tensor("hb1q_d", (ZI, K), F32, kind="ExternalInput")
    # row-sum selectors: produce rows [r; 1] (A) and [1; r] (B) at 64-65
    selwa_d = nc.dram_tensor("selwa_d", (ZI + 1, 66), F32R,
                             kind="ExternalInput")
    selwb_d = nc.dram_tensor("selwb_d", (ZI + 1, 66), F32R,
                             kind="ExternalInput")
    gb1_d = nc.dram_tensor("gb1_d", (128, 16), F32, kind="ExternalInput")
    bng_d = nc.dram_tensor("bng_d", (128, 16), F32, kind="ExternalInput")
    bnb_d = nc.dram_tensor("bnb_d", (128, 16), F32, kind="ExternalInput")
    mb1_d = nc.dram_tensor("mb1_d", (128, 8), F32, kind="ExternalInput")
    mb2_d = nc.dram_tensor("mb2_d", (128, 8), F32, kind="ExternalInput")
    mb3_d = nc.dram_tensor("mb3_d", (ZI, 1), F32, kind="ExternalInput")
    qout = nc.dram_tensor("qout", (GL * K * 4 * 128, NG), F32,
                          kind="ExternalOutput")

    with TileContext(nc) as tc:
        with (
            tc.tile_pool(name="const", bufs=1) as cp,
            tc.tile_pool(name="res", bufs=1) as rp,
            tc.tile_pool(name="ps", bufs=1, space="PSUM") as ps,
            tc.tile_pool(name="dram", bufs=1, space="DRAM") as dp,
        ):
            ident = cp.tile([128, 128], F32, tag="ident")
            make_identity(nc, ident[:])
            nla = cp.tile([128, 1], F32, tag="nla")
            nc.gpsimd.memset(nla[:], -math.log(UMAP_A))
            # shifted identity pad: idpad[i, 384 + i] = 1, else 0; slicing at
            # column 384-128*mb gives the diagonal mask of output block mb
            idpad = cp.tile([128, 896], F32, tag="idpad")
            nc.gpsimd.memset(idpad[:], 0.0)
            make_identity(nc, idpad[:, 384:512])

            gb1w = cp.tile([128, 16], F32, tag="gb1w")
            nc.sync.dma_start(gb1w[:], gb1_d[:, :])
            bngw = cp.tile([128, 16], F32, tag="bngw")
            nc.sync.dma_start(bngw[:], bng_d[:, :])
            bnbw = cp.tile([128, 16], F32, tag="bnbw")
            nc.sync.dma_start(bnbw[:], bnb_d[:, :])
            mb1w = cp.tile([128, 8], F32, tag="mb1w")
            nc.sync.dma_start(mb1w[:], mb1_d[:, :])
            mb2w = cp.tile([128, 8], F32, tag="mb2w")
            nc.sync.dma_start(mb2w[:], mb2_d[:, :])
            mb3w = cp.tile([ZI, 1], F32, tag="mb3w")
            nc.sync.dma_start(mb3w[:], mb3_d[:, :])
            hb1q = cp.tile([ZI, K], F32, tag="hb1q")
            nc.sync.dma_start(hb1q[:], hb1q_d[:, :])
            selwa = cp.tile([ZI + 1, 66], F32R, tag="selwa")
            nc.sync.dma_start(selwa[:], selwa_d[:, :])
            selwb = cp.tile([ZI + 1, 66], F32R, tag="selwb")
            nc.sync.dma_start(selwb[:], selwb_d[:, :])

            hT = [rp.tile([128, NL], F32R, tag=f"hT{fc}", name=f"hT{fc}")
                  for fc in range(4)]

            # chunked hi/lo tables: A covers local kb2 0-3, B covers 4-7
            sA = [dp.tile([4, 128, 2, 2 * D], F8, tag=f"sA{i}", name=f"sA{i}")
                  for i in range(L)]
            sB = [dp.tile([4, 128, 2, 2 * D], F8, tag=f"sB{i}", name=f"sB{i}")
                  for i in range(L)]
            tA = [dp.tile([KB2 // 2, 128, 2, 2 * D], F8, tag=f"tA{i}",
                          name=f"tA{i}", addr_space="Shared")
                  for i in range(L)]
            tB = [dp.tile([KB2 // 2, 128, 2, 2 * D], F8, tag=f"tB{i}",
                          name=f"tB{i}", addr_space="Shared")
                  for i in range(L)]
            bn_loc = [dp.tile([128, 8], F32, tag=f"bl{i}", name=f"bl{i}")
                      for i in range(L)]
            bn_glob = [dp.tile([128, 8], F32, tag=f"bg{i}", name=f"bg{i}",
                               addr_space="Shared")
                       for i in range(L)]
            war_l = dp.tile([128, 8], F32, name="war_l")
            war_g = dp.tile([128, 8], F32, name="war_g", addr_space="Shared")

            # ---------------- embedding + collective warmup ----------------
            with tc.tile_pool(name="emb", bufs=1) as ep:
                xt_sb = ep.tile([10, NL], F32R, tag="xt")
                nc.sync.dma_start(xt_sb[:], xt[:, :])
                ew_sb = ep.tile([10, D], F32R, tag="ew")
                nc.sync.dma_start(ew_sb[:], embw[:, :])
                wz = ep.tile([128, 8], F32, tag="wz")
                nc.gpsimd.memset(wz[:], 0.0)
                nc.sync.dma_start(war_l[:, :], wz[:])
                nc.gpsimd.collective_compute(
                    "AllReduce", ALU.add, ins=[war_l[:, :].opt()],
                    outs=[war_g[:, :].opt()], replica_groups=RG)
                for fc in range(4):
                    for j in range(4):
                        p = ps.tile([128, 512], F32, tag=f"b{4 + fc}")
                        nc.tensor.matmul(p[:], ew_sb[:, 128 * fc:128 * fc + 128],
                                         xt_sb[:, 512 * j:512 * j + 512],
                                         start=True, stop=True)
                        nc.vector.tensor_copy(hT[fc][:, 512 * j:512 * j + 512],
                                              p[:])

            # ---------------- GIN layers ----------------
            with tc.tile_pool(name="gin", bufs=1) as gp:
                for l in range(L):
                    w1s = gp.tile([128, 2048], F32R, tag="w1")
                    w2s = gp.tile([128, 2048], F32R, tag="w2")
                    for ic in range(4):
                        r0 = 512 * l + 128 * ic
                        nc.sync.dma_start(w1s[:, 512 * ic:512 * ic + 512],
                                          gw1[r0:r0 + 128, :])
                        nc.sync.dma_start(w2s[:, 512 * ic:512 * ic + 512],
                                          gw2[r0:r0 + 128, :])

                    mt = [gp.tile([128, NL], F32R, tag=f"mt{fc}",
                                  name=f"mt{fc}_{l}") for fc in range(4)]

                    if l == 0:
                        # layer-0 aggregation, exact: agg0 = (A.T xa) @ Wa
                        xg_sb = gp.tile([128, KB2, 2, 32], F8, tag="xg")
                        nc.sync.dma_start(xg_sb[:], xg8[:, :, :, :])
                        ewa = gp.tile([32, D], F32R, tag="ewa")
                        nc.sync.dma_start(ewa[:], embwa[:, :])
                        px = [ps.tile([32, 512], F32, tag=f"b{i}",
                                      name=f"px{i}") for i in range(4)]
                        for half in range(2):
                            for kb2 in range(KB2):
                                a8 = gp.tile([128, 2, 1024], F8, tag="a8",
                                             bufs=4)
                                eng = (nc.sync if kb2 % 2 == 0
                                       else nc.gpsimd)
                                eng.dma_start(
                                    a8[:],
                                    acm8[:, kb2, :,
                                         1024 * half:1024 * half + 1024])
                                lhs = xg_sb[:, kb2, :, :]
                                for dc in range(2):
                                    nc.tensor.matmul(
                                        px[2 * half + dc][:], lhs,
                                        a8[:, :, 512 * dc:512 * dc + 512],
                                        start=(kb2 == 0), stop=(kb2 == KB2 - 1),
                                        perf_mode=DR)
                        atxt = gp.tile([32, NL], F32R, tag="atxt")
                        for dt in range(4):
                            nc.vector.tensor_copy(
                                atxt[:, 512 * dt:512 * dt + 512], px[dt][:])
                        for d in range(4):
                            for jc in range(4):
                                pr0 = ps.tile([128, 512], F32, tag=f"b{4 + jc}")
                                nc.tensor.matmul(
                                    pr0[:], ewa[:, 128 * d:128 * d + 128],
                                    atxt[:, 512 * jc:512 * jc + 512],
                                    start=True, stop=True)
                                nc.vector.tensor_tensor(
                                    out=mt[d][:, 512 * jc:512 * jc + 512],
                                    in0=pr0[:],
                                    in1=hT[d][:, 512 * jc:512 * jc + 512],
                                    op=ALU.add)
                    else:
                        # aggregation: aggT = (hi + lo).T @ A8 (DoubleRow) + hT
                        # consume chunk-A table rows first, then chunk-B
                        order = ([("A", r) for r in range(KB2 // 2)]
                                 + [("B", r) for r in range(KB2 // 2)])
                        for half in range(2):
                            pb = [ps.tile([128, 512], F32, tag=f"b{i}",
                                          name=f"pb{i}") for i in range(8)]
                            for ii, (ab, r) in enumerate(order):
                                gkb2 = 8 * (r // 4) + (r % 4) + (4 if ab == "B"
                                                                 else 0)
                                a8 = gp.tile([128, 2, 1024], F8, tag="a8",
                                             bufs=4)
                                nc.sync.dma_start(
                                    a8[:],
                                    acm8[:, gkb2, :,
                                         1024 * half:1024 * half + 1024])
                                hk8 = gp.tile([128, 2, 1024], F8, tag="hk8",
                                              bufs=4)
                                tab = tA[l] if ab == "A" else tB[l]
                                nc.gpsimd.dma_start(hk8[:],
                                                    tab[r, :, :, :])
                                for d in range(4):
                                    for hilo in range(2):
                                        lhs = hk8[:, :,
                                                  512 * hilo + 128 * d:
                                                  512 * hilo + 128 * d + 128]
                                        for dc in range(2):
                                            nc.tensor.matmul(
                                                pb[2 * d + dc][:], lhs,
                                                a8[:, :,
                                                   512 * dc:512 * dc + 512],
                                                start=(ii == 0 and hilo == 0),
                                                stop=(ii == KB2 - 1
                                                      and hilo == 1),
                                                perf_mode=DR)
                            for d in range(4):
                                for dc in range(2):
                                    col = 1024 * half + 512 * dc
                                    nc.vector.tensor_tensor(
                                        out=mt[d][:, col:col + 512],
                                        in0=pb[2 * d + dc][:],
                                        in1=hT[d][:, col:col + 512],
                                        op=ALU.add)

                    # GIN MLP: u1 = relu(m@w1+b1); u2 = u1@w2 (into mt)
                    # fold BN partial stats in as u2 chunks are produced
                    st16s = gp.tile([128, 16], F32, tag="st16s")
                    st16q = gp.tile([128, 16], F32, tag="st16q")
                    for j in range(4):
                        ncol = 512 * j
                        u1c = [gp.tile([128, 512], F32R, tag=f"u1_{oc}", bufs=2,
                                       name=f"u1c{oc}") for oc in range(4)]
                        for oc in range(4):
                            p = ps.tile([128, 512], F32, tag=f"b{oc}")
                            for ic in range(4):
                                nc.tensor.matmul(
                                    p[:],
                                    w1s[:, 512 * ic + 128 * oc:
                                        512 * ic + 128 * oc + 128],
                                    mt[ic][:, ncol:ncol + 512],
                                    start=(ic == 0), stop=(ic == 3))
                            nc.scalar.activation(
                                u1c[oc][:], p[:], AF.Relu,
                                bias=gb1w[:, 4 * l + oc:4 * l + oc + 1])
                        for oc in range(4):
                            p = ps.tile([128, 512], F32, tag=f"b{4 + oc}")
                            for ic in range(4):
                                nc.tensor.matmul(
                                    p[:],
                                    w2s[:, 512 * ic + 128 * oc:
                                        512 * ic + 128 * oc + 128],
                                    u1c[ic][:],
                                    start=(ic == 0), stop=(ic == 3))
                            nc.vector.tensor_copy(mt[oc][:, ncol:ncol + 512],
                                                  p[:])
                            col = 4 * j + oc
                            sq = gp.tile([128, 512], F32, tag="sq", bufs=2)
                            nc.scalar.activation(sq[:], p[:], AF.Square)
                            nc.vector.reduce_sum(st16s[:, col:col + 1],
                                                 mt[oc][:, ncol:ncol + 512],
                                                 axis=AX.X)
                            nc.vector.reduce_sum(st16q[:, col:col + 1], sq[:],
                                                 axis=AX.X)

                    # assemble + AllReduce BN stats
                    stat = gp.tile([128, 8], F32, tag="stat")
                    t4a = gp.tile([128, 4], F32, tag="t4a")
                    t4b = gp.tile([128, 4], F32, tag="t4b")
                    nc.vector.tensor_tensor(out=t4a[:], in0=st16s[:, 0:4],
                                            in1=st16s[:, 4:8], op=ALU.add)
                    nc.vector.tensor_tensor(out=t4b[:], in0=st16s[:, 8:12],
                                            in1=st16s[:, 12:16], op=ALU.add)
                    nc.vector.tensor_tensor(out=stat[:, 0:4], in0=t4a[:],
                                            in1=t4b[:], op=ALU.add)
                    t4c = gp.tile([128, 4], F32, tag="t4c")
                    t4d = gp.tile([128, 4], F32, tag="t4d")
                    nc.vector.tensor_tensor(out=t4c[:], in0=st16q[:, 0:4],
                                            in1=st16q[:, 4:8], op=ALU.add)
                    nc.vector.tensor_tensor(out=t4d[:], in0=st16q[:, 8:12],
                                            in1=st16q[:, 12:16], op=ALU.add)
                    nc.vector.tensor_tensor(out=stat[:, 4:8], in0=t4c[:],
                                            in1=t4d[:], op=ALU.add)
                    nc.sync.dma_start(bn_loc[l][:, :], stat[:])
                    nc.gpsimd.collective_compute(
                        "AllReduce", ALU.add, ins=[bn_loc[l][:, :].opt()],
                        outs=[bn_glob[l][:, :].opt()], replica_groups=RG)
                    ga = gp.tile([128, 8], F32, tag="ga")
                    nc.sync.dma_start(ga[:], bn_glob[l][:, :])

                    # BN coefficients, vectorized over the 4 feature blocks
                    mu4 = gp.tile([128, 4], F32, tag="mu4")
                    nc.vector.tensor_scalar(out=mu4[:], in0=ga[:, 0:4],
                                            scalar1=1.0 / N, scalar2=None,
                                            op0=ALU.mult)
                    ex24 = gp.tile([128, 4], F32, tag="ex24")
                    nc.vector.tensor_scalar(out=ex24[:], in0=ga[:, 4:8],
                                            scalar1=1.0 / N, scalar2=None,
                                            op0=ALU.mult)
                    mu2 = gp.tile([128, 4], F32, tag="mu2")
                    nc.vector.tensor_tensor(out=mu2[:], in0=mu4[:], in1=mu4[:],
                                            op=ALU.mult)
                    var4 = gp.tile([128, 4], F32, tag="var4")
                    nc.vector.tensor_tensor(out=var4[:], in0=ex24[:],
                                            in1=mu2[:], op=ALU.subtract)
                    vare = gp.tile([128, 4], F32, tag="vare")
                    nc.vector.tensor_scalar(out=vare[:], in0=var4[:],
                                            scalar1=BN_EPS, scalar2=None,
                                            op0=ALU.add)
                    std4 = gp.tile([128, 4], F32, tag="std4")
                    nc.scalar.activation(std4[:], vare[:], AF.Sqrt)
                    inv4 = gp.tile([128, 4], F32, tag="inv4")
                    nc.vector.reciprocal(inv4[:], std4[:])
                    sv4 = gp.tile([128, 4], F32, tag="sv4")
                    nc.vector.tensor_tensor(out=sv4[:], in0=inv4[:],
                                            in1=bngw[:, 4 * l:4 * l + 4],
                                            op=ALU.mult)
                    mst = gp.tile([128, 4], F32, tag="mst")
                    nc.vector.tensor_tensor(out=mst[:], in0=mu4[:], in1=sv4[:],
                                            op=ALU.mult)
                    tv4 = gp.tile([128, 4], F32, tag="tv4")
                    nc.vector.tensor_tensor(out=tv4[:],
                                            in0=bnbw[:, 4 * l:4 * l + 4],
                                            in1=mst[:], op=ALU.subtract)

                    # BN apply + residual, j-outer so transposes start early
                    for j in range(4):
                        ncol = 512 * j
                        for fc in range(4):
                            rt = gp.tile([128, 512], F32R, tag=f"rt{fc}",
                                         bufs=2, name=f"rt{fc}")
                            nc.scalar.activation(
                                rt[:], mt[fc][:, ncol:ncol + 512], AF.Relu,
                                bias=tv4[:, fc:fc + 1], scale=sv4[:, fc:fc + 1])
                            nc.vector.tensor_tensor(
                                out=hT[fc][:, ncol:ncol + 512], in0=rt[:],
                                in1=hT[fc][:, ncol:ncol + 512], op=ALU.add)
                        if l < L - 1:
                            for nb in range(4 * j, 4 * j + 4):
                                hn2 = gp.tile([128, 2 * 512], F8, tag="hn2",
                                              bufs=2)
                                ptw = ps.tile([128, 512], F32, tag="b4")
                                for fc in range(4):
                                    nc.tensor.transpose(
                                        ptw[:, 128 * fc:128 * fc + 128],
                                        hT[fc][:, 128 * nb:128 * nb + 128]
                                        .bitcast(F32),
                                        ident[:])
                                nc.vector.tensor_copy(hn2[:, 0:512], ptw[:])
                                h32w = gp.tile([128, 512], F32, tag="h32w",
                                               bufs=2)
                                nc.vector.tensor_copy(h32w[:], hn2[:, 0:512])
                                nc.vector.tensor_tensor(
                                    out=hn2[:, 512:1024], in0=ptw[:],
                                    in1=h32w[:], op=ALU.subtract)
                                sl = sA[l + 1] if nb < 8 else sB[l + 1]
                                nc.sync.dma_start(
                                    sl[(nb % 8) // 2, :, nb % 2, :], hn2[:])
                            if j == 1:
                                nc.gpsimd.collective_compute(
                                    "AllGather", ALU.bypass,
                                    ins=[sA[l + 1][:, :, :, :].opt()],
                                    outs=[tA[l + 1][:, :, :, :].opt()],
                                    replica_groups=RG)
                            if j == 3:
                                nc.gpsimd.collective_compute(
                                    "AllGather", ALU.bypass,
                                    ins=[sB[l + 1][:, :, :, :].opt()],
                                    outs=[tB[l + 1][:, :, :, :].opt()],
                                    replica_groups=RG)

            # ------- final MLP + heads + pairwise, pipelined per graph -------
            with tc.tile_pool(name="fin", bufs=1) as fz:
                BF = mybir.dt.bfloat16
                mwa = [fz.tile([128, DFF], F32R, tag=f"mw1_{ic}", name=f"mwa{ic}")
                       for ic in range(4)]
                for ic in range(4):
                    nc.sync.dma_start(mwa[ic][:],
                                      mw1[128 * ic:128 * ic + 128, :])
                mwb = [fz.tile([128, DFF], F32R, tag=f"mw2_{ic}", name=f"mwb{ic}")
                       for ic in range(8)]
                for ic in range(8):
                    nc.sync.dma_start(mwb[ic][:],
                                      mw2[128 * ic:128 * ic + 128, :])
                mwc = [fz.tile([128, ZI], F32R, tag=f"mw3_{ic}", name=f"mwc{ic}")
                       for ic in range(8)]
                for ic in range(8):
                    nc.sync.dma_start(mwc[ic][:],
                                      mw3[128 * ic:128 * ic + 128, :])
                hw1s = fz.tile([ZI, 2 * ZI], F32R, tag="hw1s")
                nc.sync.dma_start(hw1s[:], hw1b[0:ZI, :])
                hw1s2 = fz.tile([ZI, 2 * ZI], F32R, tag="hw1s2")
                nc.sync.dma_start(hw1s2[:], hw1b[ZI:2 * ZI, :])
                hw1p = [hw1s, hw1s2]
                hw2t = [fz.tile([ZI + 1, ZI], F32R, tag=f"hw2_{k}",
                                name=f"hw2_{k}") for k in range(K)]
                for k in range(K):
                    nc.sync.dma_start(hw2t[k][:],
                                      hw2q[(ZI + 1) * k:(ZI + 1) * (k + 1), :])
                z3g = [None] * GL
                lara = {}

                def emit_mlp(g):
                    gcol = 512 * g
                    z1 = [fz.tile([128, 512], F32R, tag=f"z1_{oc}",
                                  name=f"z1_{oc}") for oc in range(8)]
                    for w in range(2):
                        for oc in range(4 * w, 4 * w + 4):
                            p = ps.tile([128, 512], F32, tag=f"b{oc % 4}")
                            for ic in range(4):
                                nc.tensor.matmul(
                                    p[:],
                                    mwa[ic][:, 128 * oc:128 * oc + 128],
                                    hT[ic][:, gcol:gcol + 512],
                                    start=(ic == 0), stop=(ic == 3))
                            nc.scalar.activation(z1[oc][:], p[:], AF.Relu,
                                                 bias=mb1w[:, oc:oc + 1])
                    z2 = [fz.tile([128, 512], F32R, tag=f"z2_{oc}",
                                  name=f"z2_{oc}") for oc in range(8)]
                    for w in range(2):
                        for oc in range(4 * w, 4 * w + 4):
                            p = ps.tile([128, 512], F32, tag=f"b{oc % 4}")
                            for ic in range(8):
                                nc.tensor.matmul(
                                    p[:],
                                    mwb[ic][:, 128 * oc:128 * oc + 128],
                                    z1[ic][:],
                                    start=(ic == 0), stop=(ic == 7))
                            nc.scalar.activation(z2[oc][:], p[:], AF.Relu,
                                                 bias=mb2w[:, oc:oc + 1])
                    pz = ps.tile([ZI, 512], F32, tag="b0")
                    for ic in range(8):
                        nc.tensor.matmul(pz[:], mwc[ic][:, 0:ZI], z2[ic][:],
                                         start=(ic == 0), stop=(ic == 7))
                    z3g[g] = rp.tile([ZI, 512], F32R, tag=f"z3_{g}",
                                     name=f"z3_{g}")
                    nc.vector.tensor_tensor(
                        out=z3g[g][:], in0=pz[:],
                        in1=mb3w[:, 0:1].to_broadcast([ZI, 512])[:],
                        op=ALU.add)

                def emit_prep(g):
                    for kp in range(2):
                        for h in range(2):
                            k = 2 * kp + h
                            p1h = ps.tile([ZI, 512], F32, tag="b4")
                            nc.tensor.matmul(
                                p1h[:], hw1p[kp][:, 64 * h:64 * h + 64],
                                z3g[g][:], start=True, stop=True)
                            h1h = fz.tile([ZI + 1, 512], F32R, tag="h1h",
                                          bufs=2)
                            nc.scalar.activation(h1h[0:ZI, :], p1h[:],
                                                 AF.Relu,
                                                 bias=hb1q[:, k:k + 1])
                            nc.vector.memset(
                                h1h[ZI:ZI + 1, :].bitcast(F32), 1.0)
                            p2h = ps.tile([ZI, 512], F32, tag="b5")
                            nc.tensor.matmul(p2h[:], hw2t[k][:], h1h[:],
                                             start=True, stop=True)
                            la = fz.tile([66, 512], F32R, tag="la", bufs=4)
                            ra = fz.tile([66, 512], F32R, tag="ra", bufs=4)
                            nc.vector.tensor_scalar(out=la[0:ZI, :],
                                                    in0=p2h[:], scalar1=-2.0,
                                                    scalar2=None, op0=ALU.mult)
                            nc.vector.tensor_copy(ra[0:ZI, :], p2h[:])
                            sqo = fz.tile([ZI + 1, 512], F32R, tag="sqo",
                                          bufs=2)
                            nc.vector.tensor_tensor(out=sqo[0:ZI, :],
                                                    in0=ra[0:ZI, :],
                                                    in1=ra[0:ZI, :],
                                                    op=ALU.mult)
                            nc.vector.memset(
                                sqo[ZI:ZI + 1, :].bitcast(F32), 1.0)
                            raugA = ps.tile([66, 512], F32, tag="b4")
                            nc.tensor.matmul(raugA[:], selwa[:], sqo[:],
                                             start=True, stop=True)
                            raugB = ps.tile([66, 512], F32, tag="b5")
                            nc.tensor.matmul(raugB[:], selwb[:], sqo[:],
                                             start=True, stop=True)
                            nc.vector.tensor_copy(la[64:66, :],
                                                  raugA[64:66, :])
                            nc.vector.tensor_copy(ra[64:66, :],
                                                  raugB[64:66, :])
                            lara[(g, kp, h)] = (la, ra)

                def emit_pd(g):
                    for kp in range(2):
                        for h in range(2):
                            la, ra = lara.pop((g, kp, h))
                            d2w = fz.tile([128, 4 * 512], F32, tag="d2w",
                                          bufs=2)
                            for mb in range(4):
                                pd = ps.tile([128, 512], F32,
                                             tag=f"b{6 + mb % 2}")
                                nc.tensor.matmul(
                                    pd[:], la[:, 128 * mb:128 * mb + 128],
                                    ra[:], start=True, stop=True)
                                nc.vector.tensor_scalar(
                                    out=d2w[:, 512 * mb:512 * mb + 512],
                                    in0=pd[:], scalar1=1e-12,
                                    scalar2=None, op0=ALU.max)
                            lnw = fz.tile([128, 4 * 512], F32, tag="lnw",
                                          bufs=2)
                            nc.scalar.activation(lnw[:], d2w[:], AF.Ln)
                            qw = fz.tile([128, 4 * 512], F32, tag="qw",
                                         bufs=2)
                            nc.scalar.activation(qw[:], lnw[:], AF.Sigmoid,
                                                 bias=nla[:, 0:1],
                                                 scale=-UMAP_B)
                            for mb in range(4):
                                qf = fz.tile([128, 512], F32, tag="qf",
                                             bufs=2)
                                off = 384 - 128 * mb
                                nc.vector.tensor_tensor(
                                    out=qf[:],
                                    in0=qw[:, 512 * mb:512 * mb + 512],
                                    in1=idpad[:, off:off + 512],
                                    op=ALU.max)
                                row = ((g * K + 2 * kp + h) * 4 + mb) * 128
                                nc.sync.dma_start(qout[row:row + 128, :],
                                                  qf[:])

                emit_mlp(0)
                for g in range(GL):
                    emit_prep(g)
                    if g + 1 < GL:
                        emit_mlp(g + 1)
                    emit_pd(g)
    nc.compile()
    return nc


def _host_prep(inputs):
    x = np.asarray(inputs["x"], np.float32)
    edge_index = np.asarray(inputs["edge_index"], np.int64)
    src, dst = edge_index[0], edge_index[1]
    hw1 = np.asarray(inputs["head_w1"], np.float32)
    hw2 = np.asarray(inputs["head_w2"], np.float32)
    hb1 = np.asarray(inputs["head_b1"], np.float32)
    hb2 = np.asarray(inputs["head_b2"], np.float32)

    hw1b = np.zeros((2 * ZI, 2 * ZI), np.float32)
    for kp in range(2):
        hw1b[ZI * kp:ZI * kp + ZI, 0:ZI] = hw1[2 * kp]
        hw1b[ZI * kp:ZI * kp + ZI, ZI:2 * ZI] = hw1[2 * kp + 1]
    hw2qv = np.zeros((K * (ZI + 1), ZI), np.float32)
    for k in range(K):
        hw2qv[(ZI + 1) * k:(ZI + 1) * k + ZI, :] = hw2[k]
        hw2qv[(ZI + 1) * k + ZI, :] = hb2[k]
    hb1qv = np.ascontiguousarray(hb1.T)  # [ZI, K]
    selwav = np.zeros((ZI + 1, 66), np.float32)
    selwav[0:ZI, 64] = 1.0   # row 64 of raugA = sum of squares = r
    selwav[ZI, 65] = 1.0     # row 65 of raugA = ones
    selwbv = np.zeros((ZI + 1, 66), np.float32)
    selwbv[ZI, 64] = 1.0     # row 64 of raugB = ones
    selwbv[0:ZI, 65] = 1.0   # row 65 of raugB = r

    shared = {
        "embw": np.ascontiguousarray(np.vstack(
            [np.asarray(inputs["emb_w"], np.float32),
             np.asarray(inputs["emb_b"], np.float32)[None, :]])),
        "gw1": np.ascontiguousarray(
            np.asarray(inputs["gin_w1"], np.float32).reshape(L * D, D)),
        "gw2": np.ascontiguousarray(
            np.asarray(inputs["gin_w2"], np.float32).reshape(L * D, D)),
        "mw1": np.ascontiguousarray(np.asarray(inputs["mlp_w1"], np.float32)),
        "mw2": np.ascontiguousarray(np.asarray(inputs["mlp_w2"], np.float32)),
        "mw3": np.ascontiguousarray(np.asarray(inputs["mlp_w3"], np.float32)),
        "hw1b": hw1b, "hw2q": hw2qv,
        "hb1q_d": hb1qv, "selwa_d": selwav, "selwb_d": selwbv,
        "gb1_d": np.ascontiguousarray(
            np.asarray(inputs["gin_b1"], np.float32)
            .reshape(L, 4, 128).transpose(2, 0, 1).reshape(128, 16)),
        "bng_d": np.ascontiguousarray(
            np.asarray(inputs["bn_g"], np.float32)
            .reshape(L, 4, 128).transpose(2, 0, 1).reshape(128, 16)),
        "bnb_d": np.ascontiguousarray(
            np.asarray(inputs["bn_b"], np.float32)
            .reshape(L, 4, 128).transpose(2, 0, 1).reshape(128, 16)),
        "mb1_d": np.ascontiguousarray(
            np.asarray(inputs["mlp_b1"], np.float32).reshape(8, 128).T),
        "mb2_d": np.ascontiguousarray(
            np.asarray(inputs["mlp_b2"], np.float32).reshape(8, 128).T),
        "mb3_d": np.ascontiguousarray(
            np.asarray(inputs["mlp_b3"], np.float32)[:, None]),
    }

    # layer-0 exact aggregation operands
    emb_w = np.asarray(inputs["emb_w"], np.float32)
    emb_b = np.asarray(inputs["emb_b"], np.float32)
    xhi = x.astype(ml_dtypes.float8_e4m3)
    xlo = (x - xhi.astype(np.float32)).astype(ml_dtypes.float8_e4m3)
    xa = np.zeros((N, 32), ml_dtypes.float8_e4m3)
    xa[:, 0:9] = xhi
    xa[:, 10:19] = xlo
    xa[:, 20] = np.float32(1.0)
    shared["xg8"] = np.ascontiguousarray(
        xa.reshape(KB2, 2, 128, 32).transpose(2, 0, 1, 3))
    ewa = np.zeros((32, D), np.float32)
    ewa[0:9] = emb_w
    ewa[10:19] = emb_w
    ewa[20] = emb_b
    shared["embwa"] = ewa

    in_maps = []
    ones_row = np.ones((1, NL), np.float32)
    for c in range(NCORES):
        lo = NL * c
        mask = (dst >= lo) & (dst < lo + NL)
        flat = src[mask] * NL + (dst[mask] - lo)
        a = np.bincount(flat, minlength=N * NL).astype(np.float32)
        # paired layout for DoubleRow: [p, kb2, two, dst]
        a = a.reshape(KB2, 2, 128, NL).transpose(2, 0, 1, 3)
        m = dict(shared)
        m["acm8"] = np.ascontiguousarray(a).astype(ml_dtypes.float8_e4m3)
        m["xt"] = np.ascontiguousarray(
            np.vstack([x[lo:lo + NL].T, ones_row]))
        in_maps.append(m)
    return in_maps


def kernel(**inputs) -> np.ndarray:
    global _NC_CACHE
    if _NC_CACHE is None:
        _NC_CACHE = build_nc()
    nc = _NC_CACHE
    in_maps = _host_prep(inputs)
    res = run_bass_kernel_spmd(nc, in_maps, core_ids=list(range(NCORES)))
    out = np.concatenate(
        [np.asarray(res.results[c]["qout"]).reshape(GL, K, NG, NG)
         for c in range(NCORES)], axis=0)
    return out

